# revision 3
# baseline (speedup 1.0000x reference)
"""Dense linear layer out = x @ W.T + b on 8 Trainium2 NeuronCores.

Strategy: data-parallel over the batch dim (8192/8 = 1024 rows per core),
W replicated. Host pre-casts both operands to bf16 and lays them out
contraction-major (xt = x_shard.T, wt = W.T) so every DMA is contiguous and
the TensorE contraction dim lands on SBUF partitions. The device kernel is a
tiled matmul: x-shard resident in SBUF (8 MB bf16), W streamed once (32 MB),
fp32 accumulation in PSUM, bias added on PSUM eviction, fp32 output.

Per-core: M=1024, K=4096, N=4096 -> 2048 matmuls of [128x128]@[128x512].
"""

import numpy as np
import ml_dtypes

B, IN, OUT = 8192, 4096, 4096
NCORES = 8
MS = B // NCORES  # 1024 batch rows per core

P = 128
NF = 512            # matmul moving free dim (one PSUM bank of fp32)
KT = IN // P        # 32 contraction tiles
MT = MS // P        # 8 stationary tiles (output partition blocks)
NS = OUT // NF      # 8 output column slabs

WT_BUFS = 6
OUT_BUFS = 4

_cache = {}


def _build():
    import concourse.mybir as mybir
    import concourse.tile as tile
    from concourse import bacc

    nc = bacc.Bacc("TRN2", target_bir_lowering=False, debug=False,
                   num_devices=NCORES)
    xt = nc.dram_tensor("xt", [IN, MS], mybir.dt.bfloat16, kind="ExternalInput")
    wt = nc.dram_tensor("wt", [IN, OUT], mybir.dt.bfloat16, kind="ExternalInput")
    bb = nc.dram_tensor("bb", [P, OUT], mybir.dt.float32, kind="ExternalInput")
    out = nc.dram_tensor("out", [MS, OUT], mybir.dt.float32, kind="ExternalOutput")

    xt_t = xt[:].rearrange("(kt p) m -> p kt m", p=P)    # [128, KT, MS]
    wt_t = wt[:].rearrange("(kt p) n -> p kt n", p=P)    # [128, KT, OUT]
    out_t = out[:].rearrange("(mt p) n -> p mt n", p=P)  # [128, MT, OUT]

    with tile.TileContext(nc) as tc:
        with (
            tc.tile_pool(name="xres", bufs=1) as xres_pool,
            tc.tile_pool(name="bias", bufs=1) as bias_pool,
            tc.tile_pool(name="wtp", bufs=WT_BUFS) as wt_pool,
            tc.tile_pool(name="psum", bufs=8, space="PSUM") as psum_pool,
            tc.tile_pool(name="outp", bufs=OUT_BUFS) as out_pool,
        ):
            # x shard resident in SBUF, one DMA per k-tile so compute can
            # start as soon as the first tiles land.
            xres = xres_pool.tile([P, KT, MS], mybir.dt.bfloat16)
            for k in range(KT):
                nc.sync.dma_start(xres[:, k], xt_t[:, k])
            bias = bias_pool.tile([P, OUT], mybir.dt.float32)
            nc.sync.dma_start(bias[:], bb[:])

            for ns in range(NS):
                nslc = slice(ns * NF, (ns + 1) * NF)
                psums = [psum_pool.tile([P, NF], mybir.dt.float32,
                                        name="ps", tag="ps")
                         for _ in range(MT)]
                for k in range(KT):
                    wt_tile = wt_pool.tile([P, NF], mybir.dt.bfloat16)
                    nc.sync.dma_start(wt_tile[:], wt_t[:, k, nslc])
                    for m in range(MT):
                        nc.tensor.matmul(
                            psums[m][:],
                            xres[:, k, m * P:(m + 1) * P],
                            wt_tile[:],
                            start=(k == 0),
                            stop=(k == KT - 1),
                        )
                for m in range(MT):
                    ot = out_pool.tile([P, NF], mybir.dt.float32)
                    nc.vector.tensor_add(ot[:], psums[m][:], bias[:, nslc])
                    nc.sync.dma_start(out_t[:, m, nslc], ot[:])

    nc.compile()
    return nc


def kernel(x, W, b):
    from concourse.bass_utils import run_bass_kernel_spmd

    nc = _cache.get("nc")
    if nc is None:
        nc = _cache["nc"] = _build()

    bf16 = ml_dtypes.bfloat16
    x = np.asarray(x, dtype=np.float32)
    W = np.asarray(W, dtype=np.float32)
    b = np.asarray(b, dtype=np.float32)

    Wt = np.ascontiguousarray(W.astype(bf16).T)                       # [IN, OUT]
    bias = np.ascontiguousarray(
        np.broadcast_to(b.astype(np.float32)[None, :], (P, OUT)))
    xb = x.astype(bf16)

    in_maps = []
    for c in range(NCORES):
        xs = np.ascontiguousarray(xb[c * MS:(c + 1) * MS].T)          # [IN, MS]
        in_maps.append({"xt": xs, "wt": Wt, "bb": bias})

    res = run_bass_kernel_spmd(nc, in_maps, list(range(NCORES)))
    return np.concatenate(
        [res.results[c]["out"] for c in range(NCORES)], axis=0)


# revision 4
# speedup vs baseline: 1.0405x; 1.0405x over previous
"""Dense linear layer out = x @ W.T + b on 8 Trainium2 NeuronCores.

Strategy: data-parallel over the batch dim (8192/8 = 1024 rows per core),
W replicated. Host pre-casts both operands to bf16 and lays them out
contraction-major (xt = x_shard.T, wt = W.T) so every DMA is contiguous and
the TensorE contraction dim lands on SBUF partitions. The device kernel is a
tiled matmul: x-shard resident in SBUF (8 MB bf16), W streamed once (32 MB),
fp32 accumulation in PSUM, bias added on PSUM eviction, fp32 output.

Per-core: M=1024, K=4096, N=4096 -> 2048 matmuls of [128x128]@[128x512].
"""

import numpy as np
import ml_dtypes

B, IN, OUT = 8192, 4096, 4096
NCORES = 8
MS = B // NCORES  # 1024 batch rows per core

P = 128
NF = 512            # matmul moving free dim (one PSUM bank of fp32)
KT = IN // P        # 32 contraction tiles
MT = MS // P        # 8 stationary tiles (output partition blocks)
NS = OUT // NF      # 8 output column slabs

WT_BUFS = 6
OUT_BUFS = 4

_cache = {}


def _build():
    import concourse.mybir as mybir
    import concourse.tile as tile
    from concourse import bacc

    nc = bacc.Bacc("TRN2", target_bir_lowering=False, debug=False,
                   num_devices=NCORES)
    xt = nc.dram_tensor("xt", [IN, MS], mybir.dt.bfloat16, kind="ExternalInput")
    wt = nc.dram_tensor("wt", [IN, OUT], mybir.dt.bfloat16, kind="ExternalInput")
    bb = nc.dram_tensor("bb", [P, OUT], mybir.dt.float32, kind="ExternalInput")
    out = nc.dram_tensor("out", [MS, OUT], mybir.dt.float32, kind="ExternalOutput")

    xt_t = xt[:].rearrange("(kt p) m -> p kt m", p=P)    # [128, KT, MS]
    wt_t = wt[:].rearrange("(kt p) n -> p kt n", p=P)    # [128, KT, OUT]
    out_t = out[:].rearrange("(mt p) n -> p mt n", p=P)  # [128, MT, OUT]

    with tile.TileContext(nc) as tc:
        with (
            tc.tile_pool(name="xres", bufs=1) as xres_pool,
            tc.tile_pool(name="bias", bufs=1) as bias_pool,
            tc.tile_pool(name="wtp", bufs=WT_BUFS) as wt_pool,
            tc.tile_pool(name="psum", bufs=8, space="PSUM") as psum_pool,
            tc.tile_pool(name="outp", bufs=OUT_BUFS) as out_pool,
        ):
            # x shard resident in SBUF. The per-k loads are interleaved into
            # the first n-slab's k-loop below so the first matmuls only wait
            # for one xt tile + one wt tile, not the whole 8 MB. wt streams
            # on the scalar HWDGE ring so it doesn't FIFO-serialize behind
            # the xt loads on the sync ring; bias takes the SWDGE path.
            xres = xres_pool.tile([P, KT, MS], mybir.dt.bfloat16)
            bias = bias_pool.tile([P, OUT], mybir.dt.float32)
            nc.gpsimd.dma_start(bias[:], bb[:])

            for ns in range(NS):
                nslc = slice(ns * NF, (ns + 1) * NF)
                psums = [psum_pool.tile([P, NF], mybir.dt.float32,
                                        name="ps", tag="ps")
                         for _ in range(MT)]
                for k in range(KT):
                    if ns == 0:
                        nc.sync.dma_start(xres[:, k], xt_t[:, k])
                    wt_tile = wt_pool.tile([P, NF], mybir.dt.bfloat16)
                    nc.scalar.dma_start(wt_tile[:], wt_t[:, k, nslc])
                    for m in range(MT):
                        nc.tensor.matmul(
                            psums[m][:],
                            xres[:, k, m * P:(m + 1) * P],
                            wt_tile[:],
                            start=(k == 0),
                            stop=(k == KT - 1),
                        )
                for m in range(MT):
                    ot = out_pool.tile([P, NF], mybir.dt.float32)
                    nc.vector.tensor_add(ot[:], psums[m][:], bias[:, nslc])
                    nc.sync.dma_start(out_t[:, m, nslc], ot[:])

    nc.compile()
    return nc


def kernel(x, W, b):
    from concourse.bass_utils import run_bass_kernel_spmd

    nc = _cache.get("nc")
    if nc is None:
        nc = _cache["nc"] = _build()

    bf16 = ml_dtypes.bfloat16
    x = np.asarray(x, dtype=np.float32)
    W = np.asarray(W, dtype=np.float32)
    b = np.asarray(b, dtype=np.float32)

    Wt = np.ascontiguousarray(W.astype(bf16).T)                       # [IN, OUT]
    bias = np.ascontiguousarray(
        np.broadcast_to(b.astype(np.float32)[None, :], (P, OUT)))
    xb = x.astype(bf16)

    in_maps = []
    for c in range(NCORES):
        xs = np.ascontiguousarray(xb[c * MS:(c + 1) * MS].T)          # [IN, MS]
        in_maps.append({"xt": xs, "wt": Wt, "bb": bias})

    res = run_bass_kernel_spmd(nc, in_maps, list(range(NCORES)))
    return np.concatenate(
        [res.results[c]["out"] for c in range(NCORES)], axis=0)
